# revision 56
# baseline (speedup 1.0000x reference)
"""Trainium2 Bass kernel for nn_CatAttention (dense_transformer).

Math (per batch b, head h):
    probs* = softmax(W_*_W)           (8,8)   ConstrainedRead selectors
    Wp     = softmax(W_pred_W)        (8,64,64)
    WK[h]  = kron(probsK[h], I64)     (512,64)    (acts on d_in)
    WQ[h]  = kron(probsQ[h], I64) @ Wp[h]
    WV[h]  = kron(probsV[h], I64)
    k,q,v  = x @ W*                   (1024,64)
    S      = k @ q.T                  (p, qi)
    attn   = softmax((log(S+1e-20) + bias(qi-p)) / 8) over valid p<=qi
    out    = attn @ v

The relative-position bias table satisfies bias(d) = (1-d)/1023 for d>=1 and
bias(0) = -2, so the softmax numerator factors as S^{1/8} * exp(p/8184) *
(per-qi factor that cancels) with a diagonal correction RHO.  S = k.q is a
sum of 64 products of probability-averaged x entries, so it lives in a
NARROW band (observed [13.1, 19.4] for the reference distribution).  Over
that band S^{1/8} = sqrt(S^{1/4}) is approximated by sqrt(a*S + b) (minimax
linear fit of S^{1/4}, then the sqrt halves the error): one ACT pass
replaces the former Ln+Exp pair, and the per-partition factors
exp(8*p/8184) fold into the activation's scale/bias vectors.  a, b are
fitted at runtime from a subsampled S range (host side); if the observed
range is too wide for the fit (rel err > 4e-3) the kernel falls back to the
exact Ln/Exp build.  The relu(1-srow/(srow+1e-10)) correction to pre[...,0]
is <= ~1e-11 against values >= ~8 and is omitted.

Sharding: core c handles batch b=c//2 and heads 4*(c%2)..4*(c%2)+3 (two
workgroups of 2 heads stacked on the partition axis).

HW notes (measured on axon-tunneled trn2, this problem):
- Plain-f32 matmul stationary loads are pathologically slow when the
  stationary CHANGES between matmuls; bf16 everywhere avoids it.  With
  bf16 stationaries, PSUM accumulation chains run at full rate, so attn@v
  is one start/stop chain per (unit, qc, jl) into a 1-bank PSUM tile
  (the old per-slot + DVE tensor_reduce scheme, -13us DVE, dated from the
  f32 era).  Chains release via one copy; normalization (reciprocal of
  the ones-column + broadcast mul) is batched per (wg, qc).
- PSUM (16KB/partition) is split into dedicated pools: scores 2x[128,1024]
  (2-ptile groups), attn@v chains 2x[128,512], proj/vaug/warm 2x[128,512];
  the former 2x[128,2048] monolith serialized the pipeline.
- Score matmuls and the act skip the left causal garbage per group (gi0
  starts at its first consumed column; gi1 full so the act span reads no
  uninitialized PSUM -- CoreSim flags that, and stale PSUM is nonneg so
  sqrt is safe either way).
- All inputs ship bf16: input DMA is a hard floor (~20us/iter at f32,
  ~10 at bf16).  Measured dead ends (all ~1-5us WORSE on HW than the cost
  model predicts): fp8 DoubleRow projections; gpsimd (Pool) tensor_tensor
  pow (software vpowf, ~120x below model -- sank the sqrt+pow^0.25 plan);
  Pool gd4 masks; draining 1/3 of act groups on DVE via the linear fit.
  The cost model does NOT model Ldweights ("TODO" in
  instruction_cost_v2.rs), so sim under-reports matmul-count-heavy stages.
- Engine balance (sim busy): PE ~29us, ACT ~24.5us (24 Sqrt activations),
  DVE ~21.5us, Pool idle; span sim ~43us, HW ~45us/iter.
- A 5th ExternalInput makes LoadExecutable fail under the axon PJRT
  plugin (keep exactly 4); f32 consts ride in GDB as (hi, lo) bf16 pairs
  (raw bitcast trips sim finite-checks).  A failed load can poison the
  device: NRT_EXEC_UNIT_UNRECOVERABLE on the next run, recovers after.
- Timing: an all-engine tc.For_i hardware loop keeps the NEFF instruction
  stream small (no fetch cliff) and lets the rep count be large enough
  that body time dominates the ~40-80ms axon invocation floor noise;
  median of paired (T_R2-T_R1)/(R2-R1) gives ~+-1us repeatability.
"""
import math
import numpy as np

BATCH, N_CTX, D_IN, N_HEADS, D_HEAD, N_VARS = 4, 1024, 512, 8, 64, 8
P = 128
NPT = N_CTX // P          # 8 p-tiles
NKT = D_IN // P           # 4 d_in tiles
QCW = 512                 # qi chunk width
NQC = N_CTX // QCW        # 2 qi chunks
INV8184 = 1.0 / (1023.0 * 8.0)
RHO = float(np.exp(np.float64(-2.0 / 8.0) - np.float64(1.0 / 1023.0 / 8.0)))
VA = D_HEAD + 1           # v columns + ones column
GDW = 2 * P + 4 * P       # gd | gd4 (4 replicas of gd[:,127:255]), bf16
EW = NPT * QCW + QCW      # e tile cols (4608): pad so the strided
                          # 4-diag-block view stays in bounds for qc1

_COMPILED = {}

# ablation switch for profiling experiments (test-only; kernel() uses 'full')
ABLATE = "full"
DRAIN_KEEP = 2   # mm2 units buffered before draining into the PE stream
ACT_SPLIT = False  # measured: splitting act groups onto DVE is ~1us slower on HW
E_BUFS = 4       # e tile pool depth


def _softmax_f32(w):
    w = np.asarray(w, dtype=np.float32)
    m = w.max(axis=-1, keepdims=True)
    e = np.exp(w - m, dtype=np.float32)
    return e / e.sum(axis=-1, keepdims=True, dtype=np.float32)


def _host_weights(W_K_W, W_Q_W, W_V_W, W_pred_W):
    """Fold ConstrainedRead + WPred into dense (d_in, 64) mats per head."""
    probsK = _softmax_f32(W_K_W)
    probsQ = _softmax_f32(W_Q_W)
    probsV = _softmax_f32(W_V_W)
    Wp = _softmax_f32(W_pred_W)
    eye = np.eye(D_HEAD, dtype=np.float32)
    WK = np.stack([np.kron(probsK[h][:, None], eye) for h in range(N_HEADS)])
    WQm = np.stack([np.kron(probsQ[h][:, None], eye) for h in range(N_HEADS)])
    WQ = np.einsum('hde,hef->hdf', WQm, Wp).astype(np.float32)
    WV = np.stack([np.kron(probsV[h][:, None], eye) for h in range(N_HEADS)])
    return WK, WQ, WV   # each (8, 512, 64)


def _fit_root8(x, WK, WQ):
    """Minimax linear fit a*S + b ~= S^{1/4} over the (subsampled, padded)
    range of S = q.k; sqrt() of it then approximates S^{1/8} with half the
    relative error.  Returns (a, b, relerr_of_root8)."""
    xs = np.asarray(x, np.float32)[:, ::4, :]          # subsample positions
    smin, smax = np.float32(np.inf), np.float32(-np.inf)
    for b in range(xs.shape[0]):
        for h in range(N_HEADS):
            k = xs[b] @ WK[h]
            q = xs[b] @ WQ[h]
            S = q @ k.T
            smin = min(smin, S.min())
            smax = max(smax, S.max())
    pad = 0.12 * (smax - smin) + 1e-6
    lo, hi = max(float(smin) - pad, 1e-6), float(smax) + pad

    def minimax(g, dginv):
        a = (g(hi) - g(lo)) / (hi - lo)
        xstar = min(max(dginv(a), lo), hi)   # tangency: g'(x*) = a
        b = 0.5 * ((g(lo) - a * lo) + (g(xstar) - a * xstar))
        maxerr = 0.5 * abs((g(xstar) - a * xstar) - (g(lo) - a * lo))
        return a, b, maxerr / g(lo)

    a4, b4, e4 = minimax(lambda t: t ** 0.25,
                         lambda s: (4.0 * s) ** (-4.0 / 3.0))
    a8, b8, e8 = minimax(lambda t: t ** 0.125,
                         lambda s: (8.0 * s) ** (-8.0 / 7.0))
    relerr = max(0.5 * e4, e8)   # sqrt halves the ACT-path relative error
    return (float(a4), float(b4), float(a8), float(b8), float(relerr))


def _stack_wg(W, h0, nh=2):
    """nh heads of (512,64) -> SBUF layout (128, 4, nh*64): [i, kt, m]."""
    s = np.concatenate([W[h0 + j] for j in range(nh)], axis=1)   # (512, nh*64)
    return np.ascontiguousarray(s.reshape(NKT, P, nh * D_HEAD).transpose(1, 0, 2))


def _gdiag():
    """GD[i,u] = h(u-127-i); h(d<0)=0, h(0)=RHO, h(d>0)=1."""
    i = np.arange(P)[:, None]
    u = np.arange(2 * P)[None, :]
    d = u - (P - 1) - i
    g = np.where(d < 0, 0.0, np.where(d == 0, RHO, 1.0))
    return np.ascontiguousarray(g.astype(np.float32))


def _pinned_bacc_cls(exact):
    """In exact mode, pin the ACT table set containing both Ln and Exp so
    the Ln<->Exp alternation does not reload function tables (~1.3us each).
    Fast mode uses only Sqrt (one table set, loaded once) -> default Bacc."""
    import concourse.bacc as bacc
    if not exact:
        return bacc.Bacc
    import concourse.mybir as mybir
    import bass_rust as _bass_rust
    from concourse.hw_specs import get_activation_tables

    class _PinnedActBacc(bacc.Bacc):
        def insert_act_table_loads(self):
            has_activation = any(
                isinstance(i, mybir.InstActivation)
                for b in self.main_func.blocks for i in b.instructions)
            if not has_activation:
                return
            tables = [
                (name, fns if name == "natural_log_exp_and_others" else set())
                for name, fns in get_activation_tables(self.m.arch).items()
            ]
            _bass_rust.insert_act_table_loads(self, tables)

    return _PinnedActBacc


def _build_nc(reps=1, barrier=True, exact=False, hwloop=False):
    import concourse.mybir as mybir
    import concourse.tile as tile
    from contextlib import ExitStack

    level = {"dmain": 0, "proj": 1, "vproj": 2, "scores": 3, "act": 4,
             "full": 5, "act2x": 4}.get(ABLATE, 5)
    score_dup = 2 if ABLATE == "act2x" else 1

    F32 = mybir.dt.float32
    F32R = mybir.dt.float32r
    BF16 = mybir.dt.bfloat16

    def rr(ap):
        return ap.bitcast(F32R)

    nc = _pinned_bacc_cls(exact)("TRN2")
    # All inputs ship as bf16 (input DMA is a hard floor of the iteration
    # time: the f32 shipment measured ~20us/iter, bf16 halves it).  The f32
    # activation scale/bias vectors ride as 4 bitcast bf16 cols of GDB; gd4
    # (4 replicas of gd[:,127:255]) is built on device.
    # NOTE: adding a 5th ExternalInput makes LoadExecutable fail under the
    # axon PJRT plugin -- keep exactly these four.
    xT_d = nc.dram_tensor("xT", (P, NKT, N_CTX), BF16, kind="ExternalInput")
    WKQ_d = nc.dram_tensor("WKQ", (2, P, NKT, 2 * P), BF16, kind="ExternalInput")
    WV_d = nc.dram_tensor("WV", (P, NKT, 4 * D_HEAD), BF16, kind="ExternalInput")
    GDB_d = nc.dram_tensor("GDB", (P, 2 * P + 8), BF16, kind="ExternalInput")
    out_d = nc.dram_tensor("out", (N_CTX, 4 * D_HEAD), F32, kind="ExternalOutput")

    LN = mybir.ActivationFunctionType.Ln
    EXP = mybir.ActivationFunctionType.Exp
    SQRT = mybir.ActivationFunctionType.Sqrt
    COPY = mybir.ActivationFunctionType.Copy

    with tile.TileContext(nc) as tc, ExitStack() as ctx:
        const_p = ctx.enter_context(tc.tile_pool(name="const", bufs=1))
        w_p = ctx.enter_context(tc.tile_pool(name="w", bufs=2))
        kq_p = ctx.enter_context(tc.tile_pool(name="kq", bufs=2))
        va_p = ctx.enter_context(tc.tile_pool(name="va", bufs=2))
        e_p = ctx.enter_context(tc.tile_pool(name="e", bufs=E_BUFS))
        z_p = ctx.enter_context(tc.tile_pool(name="z", bufs=4))
        # PSUM budget is 16KB/partition = 4096 f32 cols; split into
        # dedicated pools so score groups, attn@v chains and projections
        # pipeline instead of serializing on two monolithic 2048-col tiles:
        # scores 2x1024 + mm2 chains 2x512 + proj/vaug/warm 2x512 = 4096.
        ps_s = ctx.enter_context(tc.tile_pool(name="ps_s", bufs=2, space="PSUM"))
        ps_m = ctx.enter_context(tc.tile_pool(name="ps_m", bufs=2, space="PSUM"))
        ps_w = ctx.enter_context(tc.tile_pool(name="ps_w", bufs=2, space="PSUM"))

        gdb = const_p.tile([P, 2 * P + 8], BF16, tag="gdb")
        gd4t = const_p.tile([P, 4, P], BF16, tag="gd4")
        sbf = const_p.tile([P, 4], F32, tag="sbf")
        gd = gdb[:, 0:2 * P]
        gd4 = gd4t[:]
        # four f32 vectors ship as (hi, lo) bf16 pairs summing to the f32
        # value (raw bitcast trips sim finite-checks): sqrt-affine scale/
        # bias for ACT groups, direct-linear scale/bias for DVE groups;
        # exact mode ships (rb, 1e-20, -, -)
        sscale = sbf[:, 0:1]
        sbias = sbf[:, 1:2]
        lscale = sbf[:, 2:3]
        lbias = sbf[:, 3:4]
        eps = const_p.tile([P, 1], F32, tag="eps")
        nc.vector.memset(eps[:], 1e-20)

        mm2q = []
        zstages = {}

        def emit_mm2_jt(item):
            e, wg, hh, qc, vaug, jl = item
            key = (wg, qc)
            if key not in zstages:
                zstages[key] = z_p.tile([P, 2, 4, VA], F32, tag="zall",
                                        name=f"zall_{wg}_{qc}")
            zall = zstages[key]
            jt = qc * 4 + jl
            # attn@v as one PSUM accumulation chain per (unit, qc, jl):
            # bf16 stationaries make accumulation groups run at full rate
            # (the old per-slot + DVE-reduce scheme dated from the f32 era).
            zps = ps_m.tile([P, QCW], F32, tag="zm")
            pts = list(range(jt + 1))
            n = len(pts)
            for i, pt in enumerate(pts):
                nc.tensor.matmul(
                    zps[:, 0:VA],
                    e[:, pt * QCW + jl * P:pt * QCW + (jl + 1) * P],
                    vaug[:, pt * 4 + wg * 2 + hh, :],
                    start=(i == 0), stop=(i == n - 1))
            # release the PSUM bank with one fast copy (on ACT: DVE takes
            # a third of the act groups); normalization is batched per
            # (wg, qc) so chains don't throttle on rcp+mul
            if ACT_SPLIT:
                nc.scalar.activation(zall[:, hh, jl, :], zps[:, 0:VA], COPY)
            else:
                nc.vector.tensor_copy(zall[:, hh, jl, :], zps[:, 0:VA])
            if hh == 1 and jl == 3:
                rcpt = z_p.tile([P, 2, 4, 1], F32, tag="rcp")
                nc.vector.reciprocal(rcpt[:], zall[:, :, :, D_HEAD:VA])
                # znorm stored [p, jl, hh, c] so the out-DMA rows are
                # contiguous (hh, c) 512B spans
                znorm = z_p.tile([P, 4, 2, D_HEAD], F32, tag="znorm",
                                 name=f"znorm_{wg}_{qc}")
                nc.vector.tensor_mul(
                    znorm.rearrange("p j h c -> p h j c"),
                    zall[:, :, :, 0:D_HEAD],
                    rcpt[:].broadcast_to((P, 2, 4, D_HEAD)))
                dst = out_d[qc * QCW:(qc + 1) * QCW,
                            wg * 2 * D_HEAD:(wg + 1) * 2 * D_HEAD]
                nc.gpsimd.dma_start(
                    dst.rearrange("(j p) c -> p j c", p=P),
                    znorm.rearrange("p j h c -> p j (h c)"))
                del zstages[key]

        def drain_mm2(keep):
            while len(mm2q) > keep:
                emit_mm2_jt(mm2q.pop(0))

        def act_main(e, sps, e0, s0, width, on_dve=False):
            """Single-pass softmax numerator from PSUM scores.  1/3 of the
            groups drain on DVE (direct linear fit of S^{1/8}) so the
            score->numerator chain isn't serialized on ACT alone."""
            if exact:
                nc.scalar.activation(e[:, e0:e0 + width], sps[:, s0:s0 + width],
                                     LN, bias=eps[:])
            elif on_dve:
                nc.vector.tensor_scalar(
                    e[:, e0:e0 + width], sps[:, s0:s0 + width],
                    lscale, lbias,
                    op0=mybir.AluOpType.mult, op1=mybir.AluOpType.add)
            else:
                nc.scalar.activation(e[:, e0:e0 + width], sps[:, s0:s0 + width],
                                     SQRT, bias=sbias, scale=sscale)

        def body(rep):
          # (no PE warmup: in the hardware loop the clock stays hot across
          # iterations, and the old warmup's consume-add into gdb[0,0]
          # serialized each iteration against the tail's gd reads; removing
          # it measured ~2.7us faster)
          # wg0 weights first: the first projection needs them plus xA;
          # wg1 weights prefetch right behind so wg1 never waits
          wkq0 = w_p.tile([P, NKT, 2 * P], BF16, tag="wkq")
          nc.sync.dma_start(wkq0[:], WKQ_d[0])
          wkq1 = w_p.tile([P, NKT, 2 * P], BF16, tag="wkq")
          nc.sync.dma_start(wkq1[:], WKQ_d[1])
          # two half-tiles so ch0 compute does not falsely depend on ch1 DMA
          xA = const_p.tile([P, NKT, QCW], BF16, tag="xA")
          xB = const_p.tile([P, NKT, QCW], BF16, tag="xB")
          nc.gpsimd.dma_start(xA[:, 0:2, :], xT_d[:, 0:2, 0:QCW])
          nc.gpsimd.dma_start(xA[:, 2:NKT, :], xT_d[:, 2:NKT, 0:QCW])
          xhalf = [xA, xB]

          def xs(kt, col, width):
              t = xhalf[col // QCW]
              c = col % QCW
              return t[:, kt, c:c + width]


          wv = w_p.tile([P, NKT, 4 * D_HEAD], BF16, tag="wv")
          vaug = va_p.tile([P, NPT * 4, VA], BF16, tag="vaug")
          vaug_dma_done = [False]

          def emit_vaug(pts):
              # v projection for all 4 heads (only needed by MM2, so emitted
              # after the first k/q projections to unblock ACT sooner)
              if not vaug_dma_done[0]:
                  nc.gpsimd.dma_start(wv[:], WV_d[:])
                  # ones columns (value c_pt) don't depend on the projection
                  for pt in range(NPT):
                      c_pt = float(math.exp(P * pt * INV8184))
                      nc.vector.memset(
                          vaug[:, pt * 4:(pt + 1) * 4, D_HEAD:VA], c_pt)
                  vaug_dma_done[0] = True
              for pt in pts:
                  vps = ps_w.tile([P, QCW], F32, tag="pw")
                  for kt in range(NKT):
                      nc.tensor.matmul(vps[:, 0:4 * D_HEAD],
                                       xs(kt, pt * P, P),
                                       wv[:, kt, :],
                                       start=(kt == 0), stop=(kt == NKT - 1))
                  c_pt = float(math.exp(P * pt * INV8184))
                  nc.vector.tensor_scalar_mul(
                      vaug[:, pt * 4:(pt + 1) * 4, 0:D_HEAD],
                      vps[:, 0:4 * D_HEAD].rearrange("p (a b) -> p a b", a=4),
                      c_pt)

          gidx = [0]
          for wg in range(2):
            if wg == 0:
                wkq = wkq0
                nc.gpsimd.dma_start(xB[:, 0:2, :], xT_d[:, 0:2, QCW:N_CTX])
                nc.gpsimd.dma_start(xB[:, 2:NKT, :],
                                    xT_d[:, 2:NKT, QCW:N_CTX])
            else:
                wkq = wkq1
            wk = wkq[:, :, 0:P]
            wq = wkq[:, :, P:2 * P]

            # kT2/qT2: [128 = 2 heads x 64 dh, 1024 p/qi]
            kt2 = kq_p.tile([P, N_CTX], BF16, tag="kt2")
            qt2 = kq_p.tile([P, N_CTX], BF16, tag="qt2")

            def emit_proj(ch):
                cs = slice(ch * QCW, (ch + 1) * QCW)
                kps = ps_w.tile([P, QCW], F32, tag="pw")
                for kt in range(NKT):
                    nc.tensor.matmul(kps[:, 0:QCW], wk[:, kt, :],
                                     xs(kt, ch * QCW, QCW),
                                     start=(kt == 0), stop=(kt == NKT - 1))
                nc.vector.tensor_copy(kt2[:, cs], kps[:, 0:QCW])
                qps = ps_w.tile([P, QCW], F32, tag="pw")
                for kt in range(NKT):
                    nc.tensor.matmul(qps[:, 0:QCW], wq[:, kt, :],
                                     xs(kt, ch * QCW, QCW),
                                     start=(kt == 0), stop=(kt == NKT - 1))
                nc.vector.tensor_copy(qt2[:, cs], qps[:, 0:QCW])

            if level < 3:
                # ablation: no scores/act/mm2 — run remaining stages upfront
                if level >= 1:
                    emit_proj(0)
                    emit_proj(1)
                if level >= 2 and wg == 0:
                    emit_vaug(range(NPT))
                if level < 2 and wg == 0 and not vaug_dma_done[0]:
                    nc.sync.dma_start(wv[:], WV_d[:])
                    vaug_dma_done[0] = True
                continue

            # wg0's ch1 projections wait on the xB DMA, so defer them past the
            # first unit's scores to keep PE fed; wg1's run upfront (xB ready)
            emit_proj(0)
            first_unit = (wg == 0)
            if wg == 1:
                emit_proj(1)

            for hh in range(2):
                hb = hh * D_HEAD
                # last unit overall is (wg1,hh1): do qc1 first there so the
                # exposed tail MM2 is the small qc0 one
                qcs = [1, 0] if (wg == 1 and hh == 1) else [0, 1]
                for qc in qcs:
                    npt = (qc + 1) * NQC * 2  # active p-tiles: 4 or 8
                    qs = slice(qc * QCW, (qc + 1) * QCW)
                    e = e_p.tile([P, EW], BF16, tag="e")
                    last_unit = wg == 1 and hh == 1 and qc == 0 and level >= 5
                    if last_unit:
                        # tail overlap: flush older attn@v first, then
                        # interleave this unit's attn@v between the act ops
                        drain_mm2(0)
                    # 2-ptile score groups: one 1024-col PSUM tile (2 banks)
                    # per group so groups pipeline through the ps_s pool.
                    # The causal region for p-tile pt only needs qi >= pt*P,
                    # so both the matmul moving span and the act span skip
                    # the left garbage (stale PSUM is nonneg -> sqrt safe).
                    for g0 in range(0, npt, 2):
                        sps = ps_s.tile([P, 2 * QCW], F32, tag="ss")
                        # gi0 starts at its first consumed column; gi1 writes
                        # its full block so the act span [offs0, 1024) reads
                        # no uninitialized PSUM (fewer, larger acts beat the
                        # saved columns: the serial ACT chain dominates on HW)
                        offs = [max(0, g0 * P - qc * QCW), 0]
                        for gi in range(2):
                            pt = g0 + gi
                            off = offs[gi]
                            for _dup in range(score_dup):
                                nc.tensor.matmul(
                                    sps[:, gi * QCW + off:(gi + 1) * QCW],
                                    kt2[hb:hb + D_HEAD, pt * P:(pt + 1) * P],
                                    qt2[hb:hb + D_HEAD,
                                        qc * QCW + off:(qc + 1) * QCW],
                                    start=True, stop=True)
                        if level >= 4:
                            gidx[0] += 1
                            act_main(e, sps, g0 * QCW + offs[0], offs[0],
                                     2 * QCW - offs[0],
                                     on_dve=(ACT_SPLIT and gidx[0] % 3 == 2))
                            if exact:
                                nc.scalar.activation(
                                    e[:, g0 * QCW + offs[0]:(g0 + 2) * QCW],
                                    e[:, g0 * QCW + offs[0]:(g0 + 2) * QCW],
                                    EXP, bias=sscale, scale=0.125)
                        if last_unit:
                            for b in (g0, g0 + 1):
                                ds = slice(b * QCW + b * P,
                                           b * QCW + (b + 1) * P)
                                nc.vector.tensor_mul(e[:, ds], e[:, ds],
                                                     gd[:, P - 1:2 * P - 1])
                                emit_mm2_jt((e, wg, hh, qc, vaug, b))
                    if last_unit:
                        continue
                    if level >= 4:
                        # all 4 diagonal 128-blocks sit at stride 640 from
                        # qc*2048: one strided mul instead of four
                        ev = e[:, qc * 2048:qc * 2048 + 4 * 640].rearrange(
                            "p (a b) -> p a b", b=640)[:, :, 0:P]
                        nc.vector.tensor_mul(ev, ev, gd4)
                    # software pipeline: run the PREVIOUS unit's attn@v during
                    # this unit's act window so PE and ACT overlap
                    if first_unit:
                        emit_proj(1)
                        first_unit = False
                        if wg == 0:
                            emit_vaug(range(0, 1))
                    if wg == 0 and hh == 0 and qc == 1:
                        emit_vaug(range(1, NPT))
                    if level >= 5:
                        drain_mm2(DRAIN_KEEP)
                        for jl in range(4):
                            mm2q.append((e, wg, hh, qc, vaug, jl))
          drain_mm2(0)
          if level < 5:
              # ablation: emit equivalent out-DMA traffic from scratch tiles
              for wg in range(2):
                  for qc in range(2):
                      zst = z_p.tile([P, 4, 2 * D_HEAD], F32, tag="zst")
                      nc.vector.memset(zst[0:1, 0:1, 0:1], 0.0)
                      dst = out_d[qc * QCW:(qc + 1) * QCW,
                                  wg * 2 * D_HEAD:(wg + 1) * 2 * D_HEAD]
                      nc.sync.dma_start(dst.rearrange("(j p) c -> p j c", p=P),
                                        zst[:])

        # constants are DMA'd once, before the rep loop; gd4 replicas and
        # the f32 scale/bias are built on device (one-time, off the
        # iteration path)
        nc.sync.dma_start(gdb[:], GDB_d[:])
        for r in range(4):
            nc.vector.tensor_copy(gd4t[:, r, :], gd[:, P - 1:2 * P - 1])
        nc.vector.tensor_add(sbf[:], gdb[:, 2 * P:2 * P + 4],
                             gdb[:, 2 * P + 4:2 * P + 8])
        if hwloop and reps > 1:
            with tc.For_i(0, reps, 1):
                body(0)
        else:
            for rep in range(reps):
                if rep and barrier:
                    tc.strict_bb_all_engine_barrier()
                body(rep)
    nc.finalize()
    return nc


def _get_nc(reps=1, barrier=True, exact=False, hwloop=False):
    key = (reps, barrier, exact, hwloop)
    if key not in _COMPILED:
        _COMPILED[key] = _build_nc(reps, barrier, exact, hwloop)
    return _COMPILED[key]


def _make_runner(nc, in_maps):
    """Reusable jitted 8-core runner (no donation, device-resident inputs)."""
    import jax
    from jax.sharding import Mesh, NamedSharding, PartitionSpec
    from jax.experimental.shard_map import shard_map
    import concourse.bass2jax as b2j
    import concourse.mybir as mybir

    b2j.install_neuronx_cc_hook()
    partition_name = nc.partition_id_tensor.name if nc.partition_id_tensor else None
    in_names, out_names, out_avals, zero_outs = [], [], [], []
    for alloc in nc.m.functions[0].allocations:
        if not isinstance(alloc, mybir.MemoryLocationSet):
            continue
        name = alloc.memorylocations[0].name
        if alloc.kind == "ExternalInput":
            if name != partition_name:
                in_names.append(name)
        elif alloc.kind == "ExternalOutput":
            out_names.append(name)
            shape = tuple(alloc.tensor_shape)
            dtype = mybir.dt.np(alloc.dtype)
            out_avals.append(jax.core.ShapedArray(shape, dtype))
            zero_outs.append(np.zeros(shape, dtype))
    n_params = len(in_names)
    all_in = in_names + out_names + ([partition_name] if partition_name else [])

    def _body(*args):
        operands = list(args)
        if partition_name:
            operands.append(b2j.partition_id_tensor())
        outs = b2j._bass_exec_p.bind(
            *operands, out_avals=tuple(out_avals), in_names=tuple(all_in),
            out_names=tuple(out_names), lowering_input_output_aliases=(),
            sim_require_finite=True, sim_require_nnan=True, nc=nc)
        return tuple(outs)

    n_cores = 8
    devices = jax.devices()[:n_cores]
    mesh = Mesh(np.asarray(devices), ("core",))
    nspec = n_params + len(out_names)
    fn = jax.jit(
        shard_map(_body, mesh=mesh, in_specs=(PartitionSpec("core"),) * nspec,
                  out_specs=(PartitionSpec("core"),) * len(out_names),
                  check_rep=False),
        keep_unused=True)
    concat_in = [np.concatenate([np.asarray(in_maps[c][nm]) for c in range(n_cores)],
                                axis=0) for nm in in_names]
    concat_zeros = [np.zeros((n_cores * z.shape[0], *z.shape[1:]), z.dtype)
                    for z in zero_outs]
    sh = NamedSharding(mesh, PartitionSpec("core"))
    args = [jax.device_put(a, sh) for a in concat_in + concat_zeros]

    def run():
        outs = fn(*args)
        jax.block_until_ready(outs)
        return outs
    return run, out_names, out_avals


def _make_in_maps(x, WK, WQ, WV, exact=False):
    gdiag = _gdiag()
    if exact:
        sb = np.stack([
            np.arange(P, dtype=np.float32) * np.float32(INV8184),
            np.full(P, 1e-20, np.float32),
            np.zeros(P, np.float32),
            np.zeros(P, np.float32),
        ], axis=1)
    else:
        a4, b4, a8, b8, relerr = _fit_root8(x, WK, WQ)
        f1 = np.exp(np.arange(P, dtype=np.float64) * (8.0 * INV8184))
        f2 = f1 * f1
        sb = np.stack([
            (a4 * f2).astype(np.float32),
            (b4 * f2).astype(np.float32),
            (a8 * f1).astype(np.float32),
            (b8 * f1).astype(np.float32),
        ], axis=1)
    import ml_dtypes
    bf16 = ml_dtypes.bfloat16
    # f32 scale/bias as (hi, lo) bf16 pairs: hi + lo == f32 value
    hi = sb.astype(bf16)
    lo = (sb - hi.astype(np.float32)).astype(bf16)
    gdb = np.concatenate([gdiag.astype(bf16), hi, lo], axis=1)
    in_maps = []
    for c in range(8):
        b, hg = c // 2, c % 2
        h0 = hg * 4
        xTh = np.ascontiguousarray(x[b].T.reshape(NKT, P, N_CTX)
                                   .transpose(1, 0, 2))        # (P, NKT, 1024)
        wkq = [np.concatenate([_stack_wg(WK, h), _stack_wg(WQ, h)], axis=2)
               for h in (h0, h0 + 2)]
        in_maps.append({
            "xT": xTh.astype(bf16),
            "WKQ": np.stack(wkq).astype(bf16),
            "WV": _stack_wg(WV, h0, nh=4).astype(bf16),
            "GDB": np.ascontiguousarray(gdb),
        })
    return in_maps


def _mask_is_tril(mask):
    mask = np.asarray(mask)
    tril = np.tril(np.ones((N_CTX, N_CTX), dtype=bool))
    return all(np.array_equal(mask[b], tril) for b in range(mask.shape[0]))


def _reference_fallback(x, mask, W_K_W, W_Q_W, W_V_W, W_pred_W):
    """Exact numpy mirror of the reference for non-causal masks."""
    x = np.asarray(x, np.float32)
    mask = np.asarray(mask, bool)
    WK, WQ, WV = _host_weights(W_K_W, W_Q_W, W_V_W, W_pred_W)
    M = N_CTX
    table = np.concatenate([
        np.array([-2.0], np.float32),
        (np.linspace(0.0, -float(M), M - 1) / M).astype(np.float32),
        (np.linspace(-float(M), 0.0, M) / M).astype(np.float32)])
    rel = (np.arange(M)[None, :] - np.arange(M)[:, None]) % (2 * M)
    bias = table[rel]
    out = np.zeros((BATCH, N_CTX, N_HEADS * D_HEAD), np.float32)
    for b in range(BATCH):
        for h in range(N_HEADS):
            k = x[b] @ WK[h]
            q = x[b] @ WQ[h]
            v = x[b] @ WV[h]
            pre = q @ k.T                                   # (qi, p)
            srow = np.where(mask[b], pre, 0.0).sum(-1)
            ms = srow / (srow + 1e-10)
            pre[:, 0] += np.maximum(1.0 - ms, 0.0)
            pos = np.log(pre + 1e-20) + bias
            masked = np.where(mask[b], pos, -1e30)
            masked = masked / 8.0
            masked -= masked.max(-1, keepdims=True)
            ex = np.exp(masked)
            attn = ex / ex.sum(-1, keepdims=True)
            out[b, :, h * 64:(h + 1) * 64] = attn @ v
    return out


def _run(inputs):
    from concourse.bass_utils import run_bass_kernel_spmd
    x = np.asarray(inputs["x"], np.float32)
    WK, WQ, WV = _host_weights(inputs["W_K_W"], inputs["W_Q_W"],
                               inputs["W_V_W"], inputs["W_pred_W"])
    relerr = _fit_root8(x, WK, WQ)[4]
    exact = relerr > 4e-3     # fit unusable -> exact Ln/Exp build
    nc = _get_nc(exact=exact)
    in_maps = _make_in_maps(x, WK, WQ, WV, exact=exact)
    res = run_bass_kernel_spmd(nc, in_maps, list(range(8)))
    out = np.empty((BATCH, N_CTX, N_HEADS * D_HEAD), np.float32)
    for c in range(8):
        b, hg = c // 2, c % 2
        out[b, :, hg * 256:(hg + 1) * 256] = res.results[c]["out"]
    return out, res


def kernel(**inputs) -> np.ndarray:
    if not _mask_is_tril(inputs["mask"]):
        return _reference_fallback(**inputs)
    out, _ = _run(inputs)
    return out


# revision 57
# speedup vs baseline: 1.0388x; 1.0388x over previous
"""Trainium2 Bass kernel for nn_CatAttention (dense_transformer).

Math (per batch b, head h):
    probs* = softmax(W_*_W)           (8,8)   ConstrainedRead selectors
    Wp     = softmax(W_pred_W)        (8,64,64)
    WK[h]  = kron(probsK[h], I64)     (512,64)    (acts on d_in)
    WQ[h]  = kron(probsQ[h], I64) @ Wp[h]
    WV[h]  = kron(probsV[h], I64)
    k,q,v  = x @ W*                   (1024,64)
    S      = k @ q.T                  (p, qi)
    attn   = softmax((log(S+1e-20) + bias(qi-p)) / 8) over valid p<=qi
    out    = attn @ v

The relative-position bias table satisfies bias(d) = (1-d)/1023 for d>=1 and
bias(0) = -2, so the softmax numerator factors as S^{1/8} * exp(p/8184) *
(per-qi factor that cancels) with a diagonal correction RHO.  S = k.q is a
sum of 64 products of probability-averaged x entries, so it lives in a
NARROW band (observed [13.1, 19.4] for the reference distribution).  Over
that band S^{1/8} = sqrt(S^{1/4}) is approximated by sqrt(a*S + b) (minimax
linear fit of S^{1/4}, then the sqrt halves the error): one ACT pass
replaces the former Ln+Exp pair, and the per-partition factors
exp(8*p/8184) fold into the activation's scale/bias vectors.  a, b are
fitted at runtime from a subsampled S range (host side); if the observed
range is too wide for the fit (rel err > 4e-3) the kernel falls back to the
exact Ln/Exp build.  The relu(1-srow/(srow+1e-10)) correction to pre[...,0]
is <= ~1e-11 against values >= ~8 and is omitted.

Sharding: core c handles batch b=c//2 and heads 4*(c%2)..4*(c%2)+3 (two
workgroups of 2 heads stacked on the partition axis).

HW notes (measured on axon-tunneled trn2, this problem):
- Plain-f32 matmul stationary loads are pathologically slow when the
  stationary CHANGES between matmuls; bf16 everywhere avoids it.  With
  bf16 stationaries, PSUM accumulation chains run at full rate, so attn@v
  is one start/stop chain per (unit, qc, jl) into a 1-bank PSUM tile
  (the old per-slot + DVE tensor_reduce scheme, -13us DVE, dated from the
  f32 era).  Chains release via one copy; normalization (reciprocal of
  the ones-column + broadcast mul) is batched per (wg, qc).
- PSUM (16KB/partition) is split into dedicated pools: scores 2x[128,1024]
  (2-ptile groups), attn@v chains 2x[128,512], proj/vaug/warm 2x[128,512];
  the former 2x[128,2048] monolith serialized the pipeline.
- Score matmuls and the act skip the left causal garbage per group (gi0
  starts at its first consumed column; gi1 full so the act span reads no
  uninitialized PSUM -- CoreSim flags that, and stale PSUM is nonneg so
  sqrt is safe either way).
- All inputs ship bf16: input DMA is a hard floor (~20us/iter at f32,
  ~10 at bf16).  Measured dead ends (all ~1-5us WORSE on HW than the cost
  model predicts): fp8 DoubleRow projections; gpsimd (Pool) tensor_tensor
  pow (software vpowf, ~120x below model -- sank the sqrt+pow^0.25 plan);
  Pool gd4 masks; draining 1/3 of act groups on DVE via the linear fit.
  The cost model does NOT model Ldweights ("TODO" in
  instruction_cost_v2.rs), so sim under-reports matmul-count-heavy stages.
- Engine balance (sim busy): PE ~28us, ACT ~24.5us (24 Sqrt activations),
  DVE ~21.5us, Pool idle; HW ~47us/iter in neutral windows (machine drift
  is +-3-5us between windows; only interleaved same-window A/Bs resolve
  changes).  No per-iteration PE warmup: the loop keeps the clock hot and
  the old warmup's consume-add serialized each iteration (the sim's
  p-state model predicts the opposite; HW is truth).  wkq1 prefetches at
  body start (+1.4us); prefetching wv the same way measured 3us WORSE
  (queues ahead of the xB pieces ch1 needs) -- prefetch only critical-path
  tensors, in consumption order per queue.  Each dma_start costs ~1us
  issue overhead: coalesce, and split tensors at most once.
- A 5th ExternalInput makes LoadExecutable fail under the axon PJRT
  plugin (keep exactly 4); f32 consts ride in GDB as (hi, lo) bf16 pairs
  (raw bitcast trips sim finite-checks).  A failed load can poison the
  device: NRT_EXEC_UNIT_UNRECOVERABLE on the next run, recovers after.
- Timing: an all-engine tc.For_i hardware loop keeps the NEFF instruction
  stream small (no fetch cliff) and lets the rep count be large enough
  that body time dominates the ~40-80ms axon invocation floor noise;
  median of paired (T_R2-T_R1)/(R2-R1) gives ~+-1us repeatability.
"""
import math
import numpy as np

BATCH, N_CTX, D_IN, N_HEADS, D_HEAD, N_VARS = 4, 1024, 512, 8, 64, 8
P = 128
NPT = N_CTX // P          # 8 p-tiles
NKT = D_IN // P           # 4 d_in tiles
QCW = 512                 # qi chunk width
NQC = N_CTX // QCW        # 2 qi chunks
INV8184 = 1.0 / (1023.0 * 8.0)
RHO = float(np.exp(np.float64(-2.0 / 8.0) - np.float64(1.0 / 1023.0 / 8.0)))
VA = D_HEAD + 1           # v columns + ones column
GDW = 2 * P + 4 * P       # gd | gd4 (4 replicas of gd[:,127:255]), bf16
EW = NPT * QCW + QCW      # e tile cols (4608): pad so the strided
                          # 4-diag-block view stays in bounds for qc1

_COMPILED = {}

# ablation switch for profiling experiments (test-only; kernel() uses 'full')
ABLATE = "full"
DRAIN_KEEP = 2   # mm2 units buffered before draining into the PE stream
ACT_SPLIT = False  # measured: splitting act groups onto DVE is ~1us slower on HW
E_BUFS = 4       # e tile pool depth


def _softmax_f32(w):
    w = np.asarray(w, dtype=np.float32)
    m = w.max(axis=-1, keepdims=True)
    e = np.exp(w - m, dtype=np.float32)
    return e / e.sum(axis=-1, keepdims=True, dtype=np.float32)


def _host_weights(W_K_W, W_Q_W, W_V_W, W_pred_W):
    """Fold ConstrainedRead + WPred into dense (d_in, 64) mats per head."""
    probsK = _softmax_f32(W_K_W)
    probsQ = _softmax_f32(W_Q_W)
    probsV = _softmax_f32(W_V_W)
    Wp = _softmax_f32(W_pred_W)
    eye = np.eye(D_HEAD, dtype=np.float32)
    WK = np.stack([np.kron(probsK[h][:, None], eye) for h in range(N_HEADS)])
    WQm = np.stack([np.kron(probsQ[h][:, None], eye) for h in range(N_HEADS)])
    WQ = np.einsum('hde,hef->hdf', WQm, Wp).astype(np.float32)
    WV = np.stack([np.kron(probsV[h][:, None], eye) for h in range(N_HEADS)])
    return WK, WQ, WV   # each (8, 512, 64)


def _fit_root8(x, WK, WQ):
    """Minimax linear fit a*S + b ~= S^{1/4} over the (subsampled, padded)
    range of S = q.k; sqrt() of it then approximates S^{1/8} with half the
    relative error.  Returns (a, b, relerr_of_root8)."""
    xs = np.asarray(x, np.float32)[:, ::4, :]          # subsample positions
    smin, smax = np.float32(np.inf), np.float32(-np.inf)
    for b in range(xs.shape[0]):
        for h in range(N_HEADS):
            k = xs[b] @ WK[h]
            q = xs[b] @ WQ[h]
            S = q @ k.T
            smin = min(smin, S.min())
            smax = max(smax, S.max())
    pad = 0.12 * (smax - smin) + 1e-6
    lo, hi = max(float(smin) - pad, 1e-6), float(smax) + pad

    def minimax(g, dginv):
        a = (g(hi) - g(lo)) / (hi - lo)
        xstar = min(max(dginv(a), lo), hi)   # tangency: g'(x*) = a
        b = 0.5 * ((g(lo) - a * lo) + (g(xstar) - a * xstar))
        maxerr = 0.5 * abs((g(xstar) - a * xstar) - (g(lo) - a * lo))
        return a, b, maxerr / g(lo)

    a4, b4, e4 = minimax(lambda t: t ** 0.25,
                         lambda s: (4.0 * s) ** (-4.0 / 3.0))
    a8, b8, e8 = minimax(lambda t: t ** 0.125,
                         lambda s: (8.0 * s) ** (-8.0 / 7.0))
    relerr = max(0.5 * e4, e8)   # sqrt halves the ACT-path relative error
    return (float(a4), float(b4), float(a8), float(b8), float(relerr))


def _stack_wg(W, h0, nh=2):
    """nh heads of (512,64) -> SBUF layout (128, 4, nh*64): [i, kt, m]."""
    s = np.concatenate([W[h0 + j] for j in range(nh)], axis=1)   # (512, nh*64)
    return np.ascontiguousarray(s.reshape(NKT, P, nh * D_HEAD).transpose(1, 0, 2))


def _gdiag():
    """GD[i,u] = h(u-127-i); h(d<0)=0, h(0)=RHO, h(d>0)=1."""
    i = np.arange(P)[:, None]
    u = np.arange(2 * P)[None, :]
    d = u - (P - 1) - i
    g = np.where(d < 0, 0.0, np.where(d == 0, RHO, 1.0))
    return np.ascontiguousarray(g.astype(np.float32))


def _pinned_bacc_cls(exact):
    """In exact mode, pin the ACT table set containing both Ln and Exp so
    the Ln<->Exp alternation does not reload function tables (~1.3us each).
    Fast mode uses only Sqrt (one table set, loaded once) -> default Bacc."""
    import concourse.bacc as bacc
    if not exact:
        return bacc.Bacc
    import concourse.mybir as mybir
    import bass_rust as _bass_rust
    from concourse.hw_specs import get_activation_tables

    class _PinnedActBacc(bacc.Bacc):
        def insert_act_table_loads(self):
            has_activation = any(
                isinstance(i, mybir.InstActivation)
                for b in self.main_func.blocks for i in b.instructions)
            if not has_activation:
                return
            tables = [
                (name, fns if name == "natural_log_exp_and_others" else set())
                for name, fns in get_activation_tables(self.m.arch).items()
            ]
            _bass_rust.insert_act_table_loads(self, tables)

    return _PinnedActBacc


def _build_nc(reps=1, barrier=True, exact=False, hwloop=False):
    import concourse.mybir as mybir
    import concourse.tile as tile
    from contextlib import ExitStack

    level = {"dmain": 0, "proj": 1, "vproj": 2, "scores": 3, "act": 4,
             "full": 5, "act2x": 4}.get(ABLATE, 5)
    score_dup = 2 if ABLATE == "act2x" else 1

    F32 = mybir.dt.float32
    F32R = mybir.dt.float32r
    BF16 = mybir.dt.bfloat16

    def rr(ap):
        return ap.bitcast(F32R)

    nc = _pinned_bacc_cls(exact)("TRN2")
    # All inputs ship as bf16 (input DMA is a hard floor of the iteration
    # time: the f32 shipment measured ~20us/iter, bf16 halves it).  The f32
    # activation scale/bias vectors ride as 4 bitcast bf16 cols of GDB; gd4
    # (4 replicas of gd[:,127:255]) is built on device.
    # NOTE: adding a 5th ExternalInput makes LoadExecutable fail under the
    # axon PJRT plugin -- keep exactly these four.
    xT_d = nc.dram_tensor("xT", (P, NKT, N_CTX), BF16, kind="ExternalInput")
    WKQ_d = nc.dram_tensor("WKQ", (2, P, NKT, 2 * P), BF16, kind="ExternalInput")
    WV_d = nc.dram_tensor("WV", (P, NKT, 4 * D_HEAD), BF16, kind="ExternalInput")
    GDB_d = nc.dram_tensor("GDB", (P, 2 * P + 8), BF16, kind="ExternalInput")
    out_d = nc.dram_tensor("out", (N_CTX, 4 * D_HEAD), F32, kind="ExternalOutput")

    LN = mybir.ActivationFunctionType.Ln
    EXP = mybir.ActivationFunctionType.Exp
    SQRT = mybir.ActivationFunctionType.Sqrt
    COPY = mybir.ActivationFunctionType.Copy

    with tile.TileContext(nc) as tc, ExitStack() as ctx:
        const_p = ctx.enter_context(tc.tile_pool(name="const", bufs=1))
        w_p = ctx.enter_context(tc.tile_pool(name="w", bufs=2))
        kq_p = ctx.enter_context(tc.tile_pool(name="kq", bufs=2))
        va_p = ctx.enter_context(tc.tile_pool(name="va", bufs=2))
        e_p = ctx.enter_context(tc.tile_pool(name="e", bufs=E_BUFS))
        z_p = ctx.enter_context(tc.tile_pool(name="z", bufs=4))
        # PSUM budget is 16KB/partition = 4096 f32 cols; split into
        # dedicated pools so score groups, attn@v chains and projections
        # pipeline instead of serializing on two monolithic 2048-col tiles:
        # scores 2x1024 + mm2 chains 2x512 + proj/vaug/warm 2x512 = 4096.
        ps_s = ctx.enter_context(tc.tile_pool(name="ps_s", bufs=2, space="PSUM"))
        ps_m = ctx.enter_context(tc.tile_pool(name="ps_m", bufs=2, space="PSUM"))
        ps_w = ctx.enter_context(tc.tile_pool(name="ps_w", bufs=2, space="PSUM"))

        gdb = const_p.tile([P, 2 * P + 8], BF16, tag="gdb")
        gd4t = const_p.tile([P, 4, P], BF16, tag="gd4")
        sbf = const_p.tile([P, 4], F32, tag="sbf")
        gd = gdb[:, 0:2 * P]
        gd4 = gd4t[:]
        # four f32 vectors ship as (hi, lo) bf16 pairs summing to the f32
        # value (raw bitcast trips sim finite-checks): sqrt-affine scale/
        # bias for ACT groups, direct-linear scale/bias for DVE groups;
        # exact mode ships (rb, 1e-20, -, -)
        sscale = sbf[:, 0:1]
        sbias = sbf[:, 1:2]
        lscale = sbf[:, 2:3]
        lbias = sbf[:, 3:4]
        eps = const_p.tile([P, 1], F32, tag="eps")
        nc.vector.memset(eps[:], 1e-20)

        mm2q = []
        zstages = {}

        def emit_mm2_jt(item):
            e, wg, hh, qc, vaug, jl = item
            key = (wg, qc)
            if key not in zstages:
                zstages[key] = z_p.tile([P, 2, 4, VA], F32, tag="zall",
                                        name=f"zall_{wg}_{qc}")
            zall = zstages[key]
            jt = qc * 4 + jl
            # attn@v as one PSUM accumulation chain per (unit, qc, jl):
            # bf16 stationaries make accumulation groups run at full rate
            # (the old per-slot + DVE-reduce scheme dated from the f32 era).
            zps = ps_m.tile([P, QCW], F32, tag="zm")
            pts = list(range(jt + 1))
            n = len(pts)
            for i, pt in enumerate(pts):
                nc.tensor.matmul(
                    zps[:, 0:VA],
                    e[:, pt * QCW + jl * P:pt * QCW + (jl + 1) * P],
                    vaug[:, pt * 4 + wg * 2 + hh, :],
                    start=(i == 0), stop=(i == n - 1))
            # release the PSUM bank with one fast copy (on ACT: DVE takes
            # a third of the act groups); normalization is batched per
            # (wg, qc) so chains don't throttle on rcp+mul
            if ACT_SPLIT:
                nc.scalar.activation(zall[:, hh, jl, :], zps[:, 0:VA], COPY)
            else:
                nc.vector.tensor_copy(zall[:, hh, jl, :], zps[:, 0:VA])
            if hh == 1 and jl == 3:
                rcpt = z_p.tile([P, 2, 4, 1], F32, tag="rcp")
                nc.vector.reciprocal(rcpt[:], zall[:, :, :, D_HEAD:VA])
                # znorm stored [p, jl, hh, c] so the out-DMA rows are
                # contiguous (hh, c) 512B spans
                znorm = z_p.tile([P, 4, 2, D_HEAD], F32, tag="znorm",
                                 name=f"znorm_{wg}_{qc}")
                nc.vector.tensor_mul(
                    znorm.rearrange("p j h c -> p h j c"),
                    zall[:, :, :, 0:D_HEAD],
                    rcpt[:].broadcast_to((P, 2, 4, D_HEAD)))
                dst = out_d[qc * QCW:(qc + 1) * QCW,
                            wg * 2 * D_HEAD:(wg + 1) * 2 * D_HEAD]
                nc.gpsimd.dma_start(
                    dst.rearrange("(j p) c -> p j c", p=P),
                    znorm.rearrange("p j h c -> p j (h c)"))
                del zstages[key]

        def drain_mm2(keep):
            while len(mm2q) > keep:
                emit_mm2_jt(mm2q.pop(0))

        def act_main(e, sps, e0, s0, width, on_dve=False):
            """Single-pass softmax numerator from PSUM scores.  1/3 of the
            groups drain on DVE (direct linear fit of S^{1/8}) so the
            score->numerator chain isn't serialized on ACT alone."""
            if exact:
                nc.scalar.activation(e[:, e0:e0 + width], sps[:, s0:s0 + width],
                                     LN, bias=eps[:])
            elif on_dve:
                nc.vector.tensor_scalar(
                    e[:, e0:e0 + width], sps[:, s0:s0 + width],
                    lscale, lbias,
                    op0=mybir.AluOpType.mult, op1=mybir.AluOpType.add)
            else:
                nc.scalar.activation(e[:, e0:e0 + width], sps[:, s0:s0 + width],
                                     SQRT, bias=sbias, scale=sscale)

        def body(rep):
          # (no PE warmup: in the hardware loop the clock stays hot across
          # iterations, and the old warmup's consume-add into gdb[0,0]
          # serialized each iteration against the tail's gd reads; removing
          # it measured ~2.7us faster)
          # wg0 weights first: the first projection needs them plus xA;
          # wg1 weights prefetch right behind so wg1 never waits
          wkq0 = w_p.tile([P, NKT, 2 * P], BF16, tag="wkq")
          nc.sync.dma_start(wkq0[:], WKQ_d[0])
          wkq1 = w_p.tile([P, NKT, 2 * P], BF16, tag="wkq")
          nc.sync.dma_start(wkq1[:], WKQ_d[1])
          # two half-tiles so ch0 compute does not falsely depend on ch1 DMA
          xA = const_p.tile([P, NKT, QCW], BF16, tag="xA")
          xB = const_p.tile([P, NKT, QCW], BF16, tag="xB")
          nc.gpsimd.dma_start(xA[:, 0:2, :], xT_d[:, 0:2, 0:QCW])
          nc.gpsimd.dma_start(xA[:, 2:NKT, :], xT_d[:, 2:NKT, 0:QCW])
          xhalf = [xA, xB]

          def xs(kt, col, width):
              t = xhalf[col // QCW]
              c = col % QCW
              return t[:, kt, c:c + width]


          wv = w_p.tile([P, NKT, 4 * D_HEAD], BF16, tag="wv")
          vaug = va_p.tile([P, NPT * 4, VA], BF16, tag="vaug")
          vaug_dma_done = [False]

          def emit_vaug(pts):
              # v projection for all 4 heads (only needed by MM2, so emitted
              # after the first k/q projections to unblock ACT sooner)
              if not vaug_dma_done[0]:
                  nc.gpsimd.dma_start(wv[:], WV_d[:])
                  # ones columns (value c_pt) don't depend on the projection
                  for pt in range(NPT):
                      c_pt = float(math.exp(P * pt * INV8184))
                      nc.vector.memset(
                          vaug[:, pt * 4:(pt + 1) * 4, D_HEAD:VA], c_pt)
                  vaug_dma_done[0] = True
              for pt in pts:
                  vps = ps_w.tile([P, QCW], F32, tag="pw")
                  for kt in range(NKT):
                      nc.tensor.matmul(vps[:, 0:4 * D_HEAD],
                                       xs(kt, pt * P, P),
                                       wv[:, kt, :],
                                       start=(kt == 0), stop=(kt == NKT - 1))
                  c_pt = float(math.exp(P * pt * INV8184))
                  nc.vector.tensor_scalar_mul(
                      vaug[:, pt * 4:(pt + 1) * 4, 0:D_HEAD],
                      vps[:, 0:4 * D_HEAD].rearrange("p (a b) -> p a b", a=4),
                      c_pt)

          gidx = [0]
          for wg in range(2):
            if wg == 0:
                wkq = wkq0
                nc.gpsimd.dma_start(xB[:, 0:2, :], xT_d[:, 0:2, QCW:N_CTX])
                nc.gpsimd.dma_start(xB[:, 2:NKT, :],
                                    xT_d[:, 2:NKT, QCW:N_CTX])
            else:
                wkq = wkq1
            wk = wkq[:, :, 0:P]
            wq = wkq[:, :, P:2 * P]

            # kT2/qT2: [128 = 2 heads x 64 dh, 1024 p/qi]
            kt2 = kq_p.tile([P, N_CTX], BF16, tag="kt2")
            qt2 = kq_p.tile([P, N_CTX], BF16, tag="qt2")

            def emit_proj(ch):
                cs = slice(ch * QCW, (ch + 1) * QCW)
                kps = ps_w.tile([P, QCW], F32, tag="pw")
                for kt in range(NKT):
                    nc.tensor.matmul(kps[:, 0:QCW], wk[:, kt, :],
                                     xs(kt, ch * QCW, QCW),
                                     start=(kt == 0), stop=(kt == NKT - 1))
                nc.vector.tensor_copy(kt2[:, cs], kps[:, 0:QCW])
                qps = ps_w.tile([P, QCW], F32, tag="pw")
                for kt in range(NKT):
                    nc.tensor.matmul(qps[:, 0:QCW], wq[:, kt, :],
                                     xs(kt, ch * QCW, QCW),
                                     start=(kt == 0), stop=(kt == NKT - 1))
                nc.vector.tensor_copy(qt2[:, cs], qps[:, 0:QCW])

            if level < 3:
                # ablation: no scores/act/mm2 — run remaining stages upfront
                if level >= 1:
                    emit_proj(0)
                    emit_proj(1)
                if level >= 2 and wg == 0:
                    emit_vaug(range(NPT))
                if level < 2 and wg == 0 and not vaug_dma_done[0]:
                    nc.sync.dma_start(wv[:], WV_d[:])
                    vaug_dma_done[0] = True
                continue

            # wg0's ch1 projections wait on the xB DMA, so defer them past the
            # first unit's scores to keep PE fed; wg1's run upfront (xB ready)
            emit_proj(0)
            first_unit = (wg == 0)
            if wg == 1:
                emit_proj(1)

            for hh in range(2):
                hb = hh * D_HEAD
                # last unit overall is (wg1,hh1): do qc1 first there so the
                # exposed tail MM2 is the small qc0 one
                qcs = [1, 0] if (wg == 1 and hh == 1) else [0, 1]
                for qc in qcs:
                    npt = (qc + 1) * NQC * 2  # active p-tiles: 4 or 8
                    qs = slice(qc * QCW, (qc + 1) * QCW)
                    e = e_p.tile([P, EW], BF16, tag="e")
                    last_unit = wg == 1 and hh == 1 and qc == 0 and level >= 5
                    if last_unit:
                        # tail overlap: flush older attn@v first, then
                        # interleave this unit's attn@v between the act ops
                        drain_mm2(0)
                    # 2-ptile score groups: one 1024-col PSUM tile (2 banks)
                    # per group so groups pipeline through the ps_s pool.
                    # The causal region for p-tile pt only needs qi >= pt*P,
                    # so both the matmul moving span and the act span skip
                    # the left garbage (stale PSUM is nonneg -> sqrt safe).
                    for g0 in range(0, npt, 2):
                        sps = ps_s.tile([P, 2 * QCW], F32, tag="ss")
                        # gi0 starts at its first consumed column; gi1 writes
                        # its full block so the act span [offs0, 1024) reads
                        # no uninitialized PSUM (fewer, larger acts beat the
                        # saved columns: the serial ACT chain dominates on HW)
                        offs = [max(0, g0 * P - qc * QCW), 0]
                        for gi in range(2):
                            pt = g0 + gi
                            off = offs[gi]
                            for _dup in range(score_dup):
                                nc.tensor.matmul(
                                    sps[:, gi * QCW + off:(gi + 1) * QCW],
                                    kt2[hb:hb + D_HEAD, pt * P:(pt + 1) * P],
                                    qt2[hb:hb + D_HEAD,
                                        qc * QCW + off:(qc + 1) * QCW],
                                    start=True, stop=True)
                        if level >= 4:
                            gidx[0] += 1
                            act_main(e, sps, g0 * QCW + offs[0], offs[0],
                                     2 * QCW - offs[0],
                                     on_dve=(ACT_SPLIT and gidx[0] % 3 == 2))
                            if exact:
                                nc.scalar.activation(
                                    e[:, g0 * QCW + offs[0]:(g0 + 2) * QCW],
                                    e[:, g0 * QCW + offs[0]:(g0 + 2) * QCW],
                                    EXP, bias=sscale, scale=0.125)
                        if last_unit:
                            for b in (g0, g0 + 1):
                                ds = slice(b * QCW + b * P,
                                           b * QCW + (b + 1) * P)
                                nc.vector.tensor_mul(e[:, ds], e[:, ds],
                                                     gd[:, P - 1:2 * P - 1])
                                emit_mm2_jt((e, wg, hh, qc, vaug, b))
                    if last_unit:
                        continue
                    if level >= 4:
                        # all 4 diagonal 128-blocks sit at stride 640 from
                        # qc*2048: one strided mul instead of four
                        ev = e[:, qc * 2048:qc * 2048 + 4 * 640].rearrange(
                            "p (a b) -> p a b", b=640)[:, :, 0:P]
                        nc.vector.tensor_mul(ev, ev, gd4)
                    # software pipeline: run the PREVIOUS unit's attn@v during
                    # this unit's act window so PE and ACT overlap
                    if first_unit:
                        emit_proj(1)
                        first_unit = False
                        if wg == 0:
                            emit_vaug(range(0, 1))
                    if wg == 0 and hh == 0 and qc == 1:
                        emit_vaug(range(1, NPT))
                    if level >= 5:
                        drain_mm2(DRAIN_KEEP)
                        for jl in range(4):
                            mm2q.append((e, wg, hh, qc, vaug, jl))
          drain_mm2(0)
          if level < 5:
              # ablation: emit equivalent out-DMA traffic from scratch tiles
              for wg in range(2):
                  for qc in range(2):
                      zst = z_p.tile([P, 4, 2 * D_HEAD], F32, tag="zst")
                      nc.vector.memset(zst[0:1, 0:1, 0:1], 0.0)
                      dst = out_d[qc * QCW:(qc + 1) * QCW,
                                  wg * 2 * D_HEAD:(wg + 1) * 2 * D_HEAD]
                      nc.sync.dma_start(dst.rearrange("(j p) c -> p j c", p=P),
                                        zst[:])

        # constants are DMA'd once, before the rep loop; gd4 replicas and
        # the f32 scale/bias are built on device (one-time, off the
        # iteration path)
        nc.sync.dma_start(gdb[:], GDB_d[:])
        for r in range(4):
            nc.vector.tensor_copy(gd4t[:, r, :], gd[:, P - 1:2 * P - 1])
        nc.vector.tensor_add(sbf[:], gdb[:, 2 * P:2 * P + 4],
                             gdb[:, 2 * P + 4:2 * P + 8])
        if hwloop and reps > 1:
            with tc.For_i(0, reps, 1):
                body(0)
        else:
            for rep in range(reps):
                if rep and barrier:
                    tc.strict_bb_all_engine_barrier()
                body(rep)
    nc.finalize()
    return nc


def _get_nc(reps=1, barrier=True, exact=False, hwloop=False):
    key = (reps, barrier, exact, hwloop)
    if key not in _COMPILED:
        _COMPILED[key] = _build_nc(reps, barrier, exact, hwloop)
    return _COMPILED[key]


def _make_runner(nc, in_maps):
    """Reusable jitted 8-core runner (no donation, device-resident inputs)."""
    import jax
    from jax.sharding import Mesh, NamedSharding, PartitionSpec
    from jax.experimental.shard_map import shard_map
    import concourse.bass2jax as b2j
    import concourse.mybir as mybir

    b2j.install_neuronx_cc_hook()
    partition_name = nc.partition_id_tensor.name if nc.partition_id_tensor else None
    in_names, out_names, out_avals, zero_outs = [], [], [], []
    for alloc in nc.m.functions[0].allocations:
        if not isinstance(alloc, mybir.MemoryLocationSet):
            continue
        name = alloc.memorylocations[0].name
        if alloc.kind == "ExternalInput":
            if name != partition_name:
                in_names.append(name)
        elif alloc.kind == "ExternalOutput":
            out_names.append(name)
            shape = tuple(alloc.tensor_shape)
            dtype = mybir.dt.np(alloc.dtype)
            out_avals.append(jax.core.ShapedArray(shape, dtype))
            zero_outs.append(np.zeros(shape, dtype))
    n_params = len(in_names)
    all_in = in_names + out_names + ([partition_name] if partition_name else [])

    def _body(*args):
        operands = list(args)
        if partition_name:
            operands.append(b2j.partition_id_tensor())
        outs = b2j._bass_exec_p.bind(
            *operands, out_avals=tuple(out_avals), in_names=tuple(all_in),
            out_names=tuple(out_names), lowering_input_output_aliases=(),
            sim_require_finite=True, sim_require_nnan=True, nc=nc)
        return tuple(outs)

    n_cores = 8
    devices = jax.devices()[:n_cores]
    mesh = Mesh(np.asarray(devices), ("core",))
    nspec = n_params + len(out_names)
    fn = jax.jit(
        shard_map(_body, mesh=mesh, in_specs=(PartitionSpec("core"),) * nspec,
                  out_specs=(PartitionSpec("core"),) * len(out_names),
                  check_rep=False),
        keep_unused=True)
    concat_in = [np.concatenate([np.asarray(in_maps[c][nm]) for c in range(n_cores)],
                                axis=0) for nm in in_names]
    concat_zeros = [np.zeros((n_cores * z.shape[0], *z.shape[1:]), z.dtype)
                    for z in zero_outs]
    sh = NamedSharding(mesh, PartitionSpec("core"))
    args = [jax.device_put(a, sh) for a in concat_in + concat_zeros]

    def run():
        outs = fn(*args)
        jax.block_until_ready(outs)
        return outs
    return run, out_names, out_avals


def _make_in_maps(x, WK, WQ, WV, exact=False):
    gdiag = _gdiag()
    if exact:
        sb = np.stack([
            np.arange(P, dtype=np.float32) * np.float32(INV8184),
            np.full(P, 1e-20, np.float32),
            np.zeros(P, np.float32),
            np.zeros(P, np.float32),
        ], axis=1)
    else:
        a4, b4, a8, b8, relerr = _fit_root8(x, WK, WQ)
        f1 = np.exp(np.arange(P, dtype=np.float64) * (8.0 * INV8184))
        f2 = f1 * f1
        sb = np.stack([
            (a4 * f2).astype(np.float32),
            (b4 * f2).astype(np.float32),
            (a8 * f1).astype(np.float32),
            (b8 * f1).astype(np.float32),
        ], axis=1)
    import ml_dtypes
    bf16 = ml_dtypes.bfloat16
    # f32 scale/bias as (hi, lo) bf16 pairs: hi + lo == f32 value
    hi = sb.astype(bf16)
    lo = (sb - hi.astype(np.float32)).astype(bf16)
    gdb = np.concatenate([gdiag.astype(bf16), hi, lo], axis=1)
    in_maps = []
    for c in range(8):
        b, hg = c // 2, c % 2
        h0 = hg * 4
        xTh = np.ascontiguousarray(x[b].T.reshape(NKT, P, N_CTX)
                                   .transpose(1, 0, 2))        # (P, NKT, 1024)
        wkq = [np.concatenate([_stack_wg(WK, h), _stack_wg(WQ, h)], axis=2)
               for h in (h0, h0 + 2)]
        in_maps.append({
            "xT": xTh.astype(bf16),
            "WKQ": np.stack(wkq).astype(bf16),
            "WV": _stack_wg(WV, h0, nh=4).astype(bf16),
            "GDB": np.ascontiguousarray(gdb),
        })
    return in_maps


def _mask_is_tril(mask):
    mask = np.asarray(mask)
    tril = np.tril(np.ones((N_CTX, N_CTX), dtype=bool))
    return all(np.array_equal(mask[b], tril) for b in range(mask.shape[0]))


def _reference_fallback(x, mask, W_K_W, W_Q_W, W_V_W, W_pred_W):
    """Exact numpy mirror of the reference for non-causal masks."""
    x = np.asarray(x, np.float32)
    mask = np.asarray(mask, bool)
    WK, WQ, WV = _host_weights(W_K_W, W_Q_W, W_V_W, W_pred_W)
    M = N_CTX
    table = np.concatenate([
        np.array([-2.0], np.float32),
        (np.linspace(0.0, -float(M), M - 1) / M).astype(np.float32),
        (np.linspace(-float(M), 0.0, M) / M).astype(np.float32)])
    rel = (np.arange(M)[None, :] - np.arange(M)[:, None]) % (2 * M)
    bias = table[rel]
    out = np.zeros((BATCH, N_CTX, N_HEADS * D_HEAD), np.float32)
    for b in range(BATCH):
        for h in range(N_HEADS):
            k = x[b] @ WK[h]
            q = x[b] @ WQ[h]
            v = x[b] @ WV[h]
            pre = q @ k.T                                   # (qi, p)
            srow = np.where(mask[b], pre, 0.0).sum(-1)
            ms = srow / (srow + 1e-10)
            pre[:, 0] += np.maximum(1.0 - ms, 0.0)
            pos = np.log(pre + 1e-20) + bias
            masked = np.where(mask[b], pos, -1e30)
            masked = masked / 8.0
            masked -= masked.max(-1, keepdims=True)
            ex = np.exp(masked)
            attn = ex / ex.sum(-1, keepdims=True)
            out[b, :, h * 64:(h + 1) * 64] = attn @ v
    return out


def _run(inputs):
    from concourse.bass_utils import run_bass_kernel_spmd
    x = np.asarray(inputs["x"], np.float32)
    WK, WQ, WV = _host_weights(inputs["W_K_W"], inputs["W_Q_W"],
                               inputs["W_V_W"], inputs["W_pred_W"])
    relerr = _fit_root8(x, WK, WQ)[4]
    exact = relerr > 4e-3     # fit unusable -> exact Ln/Exp build
    nc = _get_nc(exact=exact)
    in_maps = _make_in_maps(x, WK, WQ, WV, exact=exact)
    res = run_bass_kernel_spmd(nc, in_maps, list(range(8)))
    out = np.empty((BATCH, N_CTX, N_HEADS * D_HEAD), np.float32)
    for c in range(8):
        b, hg = c // 2, c % 2
        out[b, :, hg * 256:(hg + 1) * 256] = res.results[c]["out"]
    return out, res


def kernel(**inputs) -> np.ndarray:
    if not _mask_is_tril(inputs["mask"]):
        return _reference_fallback(**inputs)
    out, _ = _run(inputs)
    return out
